# revision 14
# baseline (speedup 1.0000x reference)
"""Trainium2 Bass kernel for nn_DialogueGNNModel (gnn_message_passing).

Data-parallel over conversations: 64 conversations split 8-per-core across
8 NeuronCores. Per conversation (L=256 nodes, D=1024 features):

  1. edge attention:  scaleT[m,s] = (Wscalar.T @ x.T);  softmax over s;
     window-renormalized scores (band |m-s|<=10, off-band weight 1e-10).
  2. RGCN over the banded window graph, relations r = 4*sp[i]+2*sp[k]+dir:
     done as dense masked [L,L]@[L,H] matmuls per relation
     (mask = scores * band * dir-triangle * speaker masks).
  3. GraphConv:  h2 = Band@ (h1@W_nbr) + h1@W_self + b_gc.
  4. MatchingAttention over em=[x|h2] and classifier -> log_softmax.

All big matmuls run as float32r (FP22 single-pass, full PE rate at N>=256).
The kernel is staged in three weight-resident phases (A1: relations 0-3,
A2: relations 4-7 + root + GraphConv, C: attention + classifier) so SBUF
holds each phase's weights once while all 8 conversations stream through.
"""

import functools
from contextlib import ExitStack

import numpy as np

import concourse.bass as bass
import concourse.mybir as mybir
import concourse.tile as tile
from concourse.bass import ds
from concourse.masks import make_identity
from concourse.vector_clock import ScopedClock

L, B, D, H, R, C, WIN = 256, 64, 1024, 512, 8, 6, 10
DH = D + H
NCORES = 8
BPC = B // NCORES  # conversations per core

dt = mybir.dt
F32 = dt.float32
F32R = dt.float32r
I32 = dt.int32
AF = mybir.ActivationFunctionType
ALU = mybir.AluOpType
AX = mybir.AxisListType

DEBUG = False  # extra dram dumps of conv-0 intermediates


# ---------------------------------------------------------------------------
# Workaround: this walrus build rejects >1 sem-wait on a CTRL (Drain/Nop)
# instruction ("Too many sync wait commands").  Tile's kernel-tail drain puts
# the whole global-clock wait set on one Drain; split the waits across a
# chain of single-wait sync-engine NOPs instead.
# ---------------------------------------------------------------------------
def _drain_and_barrier_split(self, tick_clock, wait_clock):
    nc = self.nc
    probe = nc.sync.nop(nofuse=True)
    wait_clock.add_sem_waits(probe.ins, ScopedClock({None: tick_clock.global_clock}))
    si = probe.ins.sync_info
    waits = list(si.on_wait or []) if si is not None else []
    if len(waits) > 1:
        si.on_wait = waits[:1]
        for w in waits[1:]:
            n = nc.sync.nop(nofuse=True)
            n.ins.sync_info = mybir.SyncInfo(on_wait=[w], on_update=[])
    nc.sync.drain()

    nc.all_engine_barrier()
    assert self.sems is not None
    popped = nc._tile_sem_poison_stack.pop()
    assert popped is self._sem_poison
    nc.clear_and_free_semaphores(list(self.sems.allocated().values()))
    nc.all_engine_barrier()


_orig_lower_ordered = tile.TileContext._lower_ordered_insts


def _lower_ordered_split_waits(self, ordered):
    """This walrus build accepts at most ONE sem-wait per instruction.

    Tile's sem-assignment attaches every required cross-engine wait to the
    consuming instruction.  Move all but the last wait onto same-engine NOPs
    inserted just before it — the engine then blocks on each wait in turn
    before executing the instruction, which is semantically identical."""
    nc = self.nc
    for insts in ordered.values():
        out = []
        for inst in insts:
            si = inst.sync_info
            if si is not None and si.on_wait and len(si.on_wait) > 1:
                waits = list(si.on_wait)
                si.on_wait = [waits[-1]]
                for w in waits[:-1]:
                    nop = mybir.InstNoOp(
                        name=f"I-{nc.next_id()}", ins=[], outs=[],
                        engine=inst.engine, bass_nofuse=True)
                    nop.sync_info = mybir.SyncInfo(on_wait=[w], on_update=[])
                    out.append(nop)
            out.append(inst)
        insts[:] = out
    return _orig_lower_ordered(self, ordered)


def _install_patches():
    tile.TileContext._drain_and_barrier = _drain_and_barrier_split
    tile.TileContext._lower_ordered_insts = _lower_ordered_split_waits


def _r(ap):
    """Reinterpret an fp32 AP as float32r for full-rate PE matmuls."""
    return ap.bitcast(F32R)


class _CopyBalancer:
    """Alternate PSUM->SBUF evacuation copies between DVE and ACT."""

    def __init__(self, nc):
        self.nc = nc
        self.i = 0

    def copy(self, out, in_):
        if self.i % 2 == 0:
            self.nc.vector.tensor_copy(out=out, in_=in_)
        else:
            self.nc.scalar.copy(out=out, in_=in_)
        self.i += 1


def _build_nc():
    _install_patches()
    nc = bass.Bass("TRN2", target_bir_lowering=False, debug=False,
                   num_devices=NCORES)

    feat = nc.dram_tensor("features", [L, BPC, D], F32, kind="ExternalInput").ap()
    spk = nc.dram_tensor("speakers", [L, BPC], I32, kind="ExternalInput").ap()
    wsc_d = nc.dram_tensor("Wscalar", [D, L], F32, kind="ExternalInput").ap()
    wrel_d = nc.dram_tensor("W_rel", [R, D, H], F32, kind="ExternalInput").ap()
    wroot_d = nc.dram_tensor("W_root", [D, H], F32, kind="ExternalInput").ap()
    brg_d = nc.dram_tensor("b_rgcn", [H], F32, kind="ExternalInput").ap()
    wnbr_d = nc.dram_tensor("W_nbr", [H, H], F32, kind="ExternalInput").ap()
    wself_d = nc.dram_tensor("W_self", [H, H], F32, kind="ExternalInput").ap()
    bgc_d = nc.dram_tensor("b_gc", [H], F32, kind="ExternalInput").ap()
    wm_d = nc.dram_tensor("W_match", [DH, DH], F32, kind="ExternalInput").ap()
    bm_d = nc.dram_tensor("b_match", [DH], F32, kind="ExternalInput").ap()
    wlin_d = nc.dram_tensor("W_lin", [DH, H], F32, kind="ExternalInput").ap()
    blin_d = nc.dram_tensor("b_lin", [H], F32, kind="ExternalInput").ap()
    wfc_d = nc.dram_tensor("W_fc", [H, C], F32, kind="ExternalInput").ap()
    bfc_d = nc.dram_tensor("b_fc", [C], F32, kind="ExternalInput").ap()

    out_d = nc.dram_tensor("out", [BPC * L, C], F32, kind="ExternalOutput").ap()

    aggp_d = nc.dram_tensor("aggp_scratch", [BPC, 2, 2, 128, H], F32).ap()
    h1s_d = nc.dram_tensor("h1_scratch", [BPC, 2, 128, H], F32R).ap()
    h2s_d = nc.dram_tensor("h2_scratch", [BPC, 2, 128, H], F32R).ap()

    dbg = {}
    if DEBUG:
        for name, shape in [
            ("dbg_u", [128, 2, L]), ("dbg_d", [128, 2, 1]),
            ("dbg_h1", [128, 2, H]), ("dbg_h2", [128, 2, H]),
            ("dbg_att", [128, 12, L]), ("dbg_a", [128, 2, L]),
        ]:
            dbg[name] = nc.dram_tensor(name, shape, F32, kind="ExternalOutput").ap()

    with tile.TileContext(nc) as tc:
        _body(tc, feat, spk, wsc_d, wrel_d, wroot_d, brg_d, wnbr_d, wself_d,
              bgc_d, wm_d, bm_d, wlin_d, blin_d, wfc_d, bfc_d, out_d,
              aggp_d, h1s_d, h2s_d, dbg)
    return nc


def _load_xT(nc, cb, xpool, xTpool, psT, feat, j, ident):
    """DMA x (conv j) and produce its transpose xT via PE transposes.

    Returns (x_t [128, 2, 1024], xT_t [128, 8, 256])."""
    x_t = xpool.tile([128, 2, D], F32R, tag="x")
    for mb in range(2):
        nc.sync.dma_start(out=x_t[:, mb, :],
                          in_=feat[ds(mb * 128, 128), j, :].bitcast(F32R))
    xT_t = xTpool.tile([128, 8, L], F32R, tag="xT")
    for kt in range(8):
        for mb in range(2):
            tp = psT.tile([128, 128], F32R, tag="psT")
            nc.tensor.transpose(out=tp, in_=x_t[:, mb, ds(kt * 128, 128)],
                                identity=ident)
            cb.copy(out=xT_t[:, kt, ds(mb * 128, 128)], in_=tp)
    return x_t, xT_t


def _body(tc, feat, spk, wsc_d, wrel_d, wroot_d, brg_d, wnbr_d, wself_d,
          bgc_d, wm_d, bm_d, wlin_d, blin_d, wfc_d, bfc_d, out_d,
          aggp_d, h1s_d, h2s_d, dbg):
    nc = tc.nc
    cb = _CopyBalancer(nc)

    with ExitStack() as S:
        consts = S.enter_context(tc.tile_pool(name="consts", bufs=1))

        identf = consts.tile([128, 128], F32)
        make_identity(nc, identf)
        ident = consts.tile([128, 128], F32R)
        nc.vector.tensor_copy(out=ident, in_=identf)

        # Window masks. Layout [p, mb, s]: row = node i = mb*128+p, col = s.
        winm = consts.tile([128, 2, L], F32)   # 1.0 in-band else 1e-10
        band0f = consts.tile([128, 2, L], F32)  # 1.0 in-band else 0.0
        band0 = consts.tile([128, 2, L], F32R)
        btri = consts.tile([128, 2, 2, L], F32)  # [p, dbit, mb, s]
        nc.gpsimd.memset(winm, 1.0)
        nc.gpsimd.memset(band0f, 1.0)
        for mb in range(2):
            for fill, t in ((1e-10, winm), (0.0, band0f)):
                nc.gpsimd.affine_select(
                    out=t[:, mb, :], in_=t[:, mb, :], compare_op=ALU.is_ge,
                    fill=fill, base=mb * 128 + WIN, channel_multiplier=1,
                    pattern=[[-1, L]])
                nc.gpsimd.affine_select(
                    out=t[:, mb, :], in_=t[:, mb, :], compare_op=ALU.is_ge,
                    fill=fill, base=WIN - mb * 128, channel_multiplier=-1,
                    pattern=[[1, L]])
            # dbit=0: i < s ;  dbit=1: i >= s
            nc.vector.tensor_copy(out=btri[:, 0, mb, :], in_=band0f[:, mb, :])
            nc.gpsimd.affine_select(
                out=btri[:, 0, mb, :], in_=btri[:, 0, mb, :],
                compare_op=ALU.is_gt, fill=0.0, base=-mb * 128,
                channel_multiplier=-1, pattern=[[1, L]])
            nc.vector.tensor_copy(out=btri[:, 1, mb, :], in_=band0f[:, mb, :])
            nc.gpsimd.affine_select(
                out=btri[:, 1, mb, :], in_=btri[:, 1, mb, :],
                compare_op=ALU.is_ge, fill=0.0, base=mb * 128,
                channel_multiplier=1, pattern=[[-1, L]])
        nc.vector.tensor_copy(out=band0, in_=band0f)

        # biases
        brg_row = consts.tile([1, H], F32R)
        bgc_row = consts.tile([1, H], F32R)
        bfc_row = consts.tile([1, C], F32R)
        ones1f = consts.tile([1, 128], F32)
        ones1 = consts.tile([1, 128], F32R)
        bm_col = consts.tile([128, 12], F32)
        blin_col = consts.tile([128, 4], F32)
        nc.gpsimd.memset(ones1f, 1.0)
        nc.vector.tensor_copy(out=ones1, in_=ones1f)
        nc.sync.dma_start(out=brg_row, in_=brg_d.unsqueeze(0).bitcast(F32R))
        nc.sync.dma_start(out=bgc_row, in_=bgc_d.unsqueeze(0).bitcast(F32R))
        nc.sync.dma_start(out=bfc_row, in_=bfc_d.unsqueeze(0).bitcast(F32R))
        nc.sync.dma_start(out=bm_col, in_=bm_d.rearrange("(blk p) -> p blk", p=128))
        nc.sync.dma_start(out=blin_col, in_=blin_d.rearrange("(blk p) -> p blk", p=128))

        # speakers, int32 [L, BPC] -> [128, mb, BPC]
        spk_i = consts.tile([128, 2, BPC], I32)
        for mb in range(2):
            nc.sync.dma_start(out=spk_i[:, mb, :], in_=spk[ds(mb * 128, 128), :])

        with ExitStack() as SP:
            pers = SP.enter_context(tc.tile_pool(name="pers", bufs=1))
            # persistent per-conversation state (A1 -> A2)
            u_all = pers.tile([128, BPC, 2, L], F32)      # exp(scale - max)
            spm_all = pers.tile([128, BPC, 2, 2, 1], F32)  # [p, j, a, mb, 1]
            rd_all = pers.tile([128, BPC, 2, 1], F32)      # 1/denominator

            # --------------------------------------------------------------
            # Phase A1: scale/softmax/scores + RGCN relations 0-3 (a=0)
            # --------------------------------------------------------------
            with ExitStack() as P:
                wp = P.enter_context(tc.tile_pool(name="wA1", bufs=1))
                wsc = wp.tile([128, 8, L], F32R)
                nc.sync.dma_start(
                    out=wsc,
                    in_=wsc_d.rearrange("(kt p) m -> p kt m", p=128).bitcast(F32R))
                wrelA = wp.tile([128, 4, 8, H], F32R)
                for ri in range(4):
                    nc.sync.dma_start(
                        out=wrelA[:, ri],
                        in_=wrel_d[ri].rearrange("(kt p) h -> p kt h", p=128).bitcast(F32R))

                xpool = P.enter_context(tc.tile_pool(name="xA1", bufs=2))
                xTpool = P.enter_context(tc.tile_pool(name="xTA1", bufs=2))
                tiny = P.enter_context(tc.tile_pool(name="tinyA1", bufs=4))
                mskp = P.enter_context(tc.tile_pool(name="mskA1", bufs=2))
                hallsb = P.enter_context(tc.tile_pool(name="hallsbA1", bufs=4))
                aggsb = P.enter_context(tc.tile_pool(name="aggsbA1", bufs=2))
                scr = P.enter_context(tc.tile_pool(name="scrA1", bufs=2))

                psT = P.enter_context(tc.tile_pool(name="psTA1", bufs=2, space="PSUM"))
                psHall = P.enter_context(tc.tile_pool(name="psHallA1", bufs=2, space="PSUM"))
                psAgg = P.enter_context(tc.tile_pool(name="psAggA1", bufs=1, space="PSUM"))

                for j in range(BPC):
                    x_t, xT_t = _load_xT(nc, cb, xpool, xTpool, psT, feat, j, ident)

                    # --- speaker masks for conv j ---
                    for mb in range(2):
                        spf = tiny.tile([128, 1], F32, tag="spf")
                        nc.vector.tensor_copy(out=spf, in_=spk_i[:, mb, ds(j, 1)])
                        nc.vector.tensor_copy(out=spm_all[:, j, 1, mb, :], in_=spf)
                        nc.vector.tensor_scalar(
                            out=spm_all[:, j, 0, mb, :], in0=spf, scalar1=-1.0,
                            scalar2=1.0, op0=ALU.mult, op1=ALU.add)

                    # --- scale + softmax-over-s + window denominator ---
                    for mb in range(2):
                        sc = psHall.tile([128, L], F32, tag="hallp")
                        for kt in range(8):
                            nc.tensor.matmul(
                                out=sc, lhsT=_r(wsc[:, kt, ds(mb * 128, 128)]),
                                rhs=_r(xT_t[:, kt, :]),
                                start=(kt == 0), stop=(kt == 7))
                        nm = tiny.tile([128, 1], F32, tag="nm")
                        nc.vector.tensor_reduce(out=nm, in_=sc, axis=AX.X,
                                                op=ALU.max, negate=True)
                        u_ap = u_all[:, j, mb, :]
                        nc.scalar.activation(u_ap, sc, AF.Exp, bias=nm, scale=1.0)
                        tmp = scr.tile([128, L], F32, tag="scr")
                        nc.vector.tensor_tensor(out=tmp, in0=u_ap,
                                                in1=winm[:, mb, :], op=ALU.mult)
                        dd = tiny.tile([128, 1], F32, tag="dd")
                        nc.vector.tensor_reduce(out=dd, in_=tmp, axis=AX.X,
                                                op=ALU.add)
                        nc.vector.reciprocal(rd_all[:, j, mb, :], dd)

                    if DEBUG and j == 0 and "dbg_u" in dbg:
                        nc.scalar.dma_start(out=dbg["dbg_u"], in_=u_all[:, 0])
                        nc.scalar.dma_start(out=dbg["dbg_d"], in_=rd_all[:, 0])

                    _rgcn_half(nc, cb, tc, tiny, mskp, hallsb, psHall, psAgg,
                               aggsb, u_all, spm_all, rd_all, btri, xT_t, wrelA,
                               j, a=0, aggp_d=aggp_d, store=True)

            # --------------------------------------------------------------
            # Phase A2: relations 4-7 (a=1) + W_root -> h1 to scratch
            # --------------------------------------------------------------
            with ExitStack() as P:
                wp = P.enter_context(tc.tile_pool(name="wA2", bufs=1))
                wrelB = wp.tile([128, 4, 8, H], F32R)
                for ri in range(4):
                    nc.sync.dma_start(
                        out=wrelB[:, ri],
                        in_=wrel_d[4 + ri].rearrange("(kt p) h -> p kt h", p=128).bitcast(F32R))
                wroot = wp.tile([128, 8, H], F32R)
                nc.sync.dma_start(
                    out=wroot,
                    in_=wroot_d.rearrange("(kt p) h -> p kt h", p=128).bitcast(F32R))

                xpool = P.enter_context(tc.tile_pool(name="xA2", bufs=2))
                xTpool = P.enter_context(tc.tile_pool(name="xTA2", bufs=2))
                tiny = P.enter_context(tc.tile_pool(name="tinyA2", bufs=4))
                mskp = P.enter_context(tc.tile_pool(name="mskA2", bufs=2))
                hallsb = P.enter_context(tc.tile_pool(name="hallsbA2", bufs=4))
                rootsb = P.enter_context(tc.tile_pool(name="rootA2", bufs=1))
                apsb = P.enter_context(tc.tile_pool(name="apsbA2", bufs=1))
                h1p = P.enter_context(tc.tile_pool(name="h1A2", bufs=2))
                scr = P.enter_context(tc.tile_pool(name="scrA2", bufs=2))

                psT = P.enter_context(tc.tile_pool(name="psTA2", bufs=2, space="PSUM"))
                psHall = P.enter_context(tc.tile_pool(name="psHallA2", bufs=2, space="PSUM"))
                psAgg = P.enter_context(tc.tile_pool(name="psAggA2", bufs=1, space="PSUM"))

                for j in range(BPC):
                    x_t, xT_t = _load_xT(nc, cb, xpool, xTpool, psT, feat, j, ident)

                    # root transform x @ W_root
                    root_t = rootsb.tile([128, 2, H], F32, tag="root")
                    for mb in range(2):
                        rp = psHall.tile([128, H], F32, tag="hallp")
                        for kt in range(8):
                            nc.tensor.matmul(
                                out=rp, lhsT=_r(xT_t[:, kt, ds(mb * 128, 128)]),
                                rhs=_r(wroot[:, kt, :]),
                                start=(kt == 0), stop=False)
                        nc.tensor.matmul(out=rp, lhsT=ones1, rhs=brg_row,
                                         start=False, stop=True)
                        cb.copy(out=root_t[:, mb, :], in_=rp)

                    agg_ps = _rgcn_half(nc, cb, tc, tiny, mskp, hallsb, psHall,
                                        psAgg, None, u_all, spm_all, rd_all,
                                        btri, xT_t, wrelB, j, a=1, aggp_d=None,
                                        store=False)

                    # partial sums from A1
                    ap_t = apsb.tile([128, 2, 2, H], F32, tag="apsb")
                    for c in range(2):
                        for mb in range(2):
                            nc.sync.dma_start(out=ap_t[:, c, mb, :],
                                              in_=aggp_d[j, c, mb])

                    # h1 = sel(agg) + root + b_rgcn  -> h1 scratch
                    h1_t = h1p.tile([128, 2, H], F32R, tag="h1")
                    for mb in range(2):
                        t0 = scr.tile([128, H], F32, tag="t0")
                        t1 = scr.tile([128, H], F32, tag="t1")
                        nc.vector.tensor_tensor(out=t0, in0=agg_ps[0][mb],
                                                in1=ap_t[:, 0, mb, :], op=ALU.add)
                        nc.vector.tensor_tensor(out=t1, in0=agg_ps[1][mb],
                                                in1=ap_t[:, 1, mb, :], op=ALU.add)
                        nc.vector.tensor_scalar(out=t0, in0=t0,
                                                scalar1=spm_all[:, j, 0, mb, :],
                                                scalar2=None, op0=ALU.mult)
                        nc.vector.tensor_scalar(out=t1, in0=t1,
                                                scalar1=spm_all[:, j, 1, mb, :],
                                                scalar2=None, op0=ALU.mult)
                        nc.vector.tensor_tensor(out=h1_t[:, mb, :], in0=t0,
                                                in1=t1, op=ALU.add)
                        nc.vector.tensor_tensor(out=h1_t[:, mb, :],
                                                in0=h1_t[:, mb, :],
                                                in1=root_t[:, mb, :], op=ALU.add)
                        nc.scalar.dma_start(out=h1s_d[j, mb], in_=h1_t[:, mb, :])

                    if DEBUG and j == 0 and "dbg_h1" in dbg:
                        nc.scalar.dma_start(out=dbg["dbg_h1"], in_=h1_t)

        # ------------------------------------------------------------------
        # Phase B: GraphConv  h2 = h1@W_self + Band@(h1@W_nbr) + b_gc
        # ------------------------------------------------------------------
        with ExitStack() as P:
            wp = P.enter_context(tc.tile_pool(name="wB", bufs=1))
            wnbr = wp.tile([128, 4, H], F32R)
            nc.sync.dma_start(
                out=wnbr,
                in_=wnbr_d.rearrange("(kt p) h -> p kt h", p=128).bitcast(F32R))
            wself = wp.tile([128, 4, H], F32R)
            nc.sync.dma_start(
                out=wself,
                in_=wself_d.rearrange("(kt p) h -> p kt h", p=128).bitcast(F32R))

            h1pool = P.enter_context(tc.tile_pool(name="h1B", bufs=2))
            h1Tp = P.enter_context(tc.tile_pool(name="h1TB", bufs=2))
            g1p = P.enter_context(tc.tile_pool(name="g1B", bufs=2))
            h2p = P.enter_context(tc.tile_pool(name="h2B", bufs=2))

            psT = P.enter_context(tc.tile_pool(name="psTB", bufs=2, space="PSUM"))
            psB = P.enter_context(tc.tile_pool(name="psB", bufs=2, space="PSUM"))

            for j in range(BPC):
                h1_t = h1pool.tile([128, 2, H], F32R, tag="h1")
                for mb in range(2):
                    nc.sync.dma_start(out=h1_t[:, mb, :], in_=h1s_d[j, mb])

                # h1T [128, jj, 256]
                h1T_t = h1Tp.tile([128, 4, L], F32R, tag="h1T")
                for jj in range(4):
                    for mb in range(2):
                        tp = psT.tile([128, 128], F32R, tag="psT")
                        nc.tensor.transpose(
                            out=tp, in_=h1_t[:, mb, ds(jj * 128, 128)],
                            identity=ident)
                        cb.copy(out=h1T_t[:, jj, ds(mb * 128, 128)], in_=tp)

                # g1 = h1 @ W_nbr
                g1_t = g1p.tile([128, 2, H], F32R, tag="g1")
                for ib in range(2):
                    gp = psB.tile([128, H], F32, tag="psb")
                    for jj in range(4):
                        nc.tensor.matmul(
                            out=gp, lhsT=_r(h1T_t[:, jj, ds(ib * 128, 128)]),
                            rhs=_r(wnbr[:, jj, :]),
                            start=(jj == 0), stop=(jj == 3))
                    cb.copy(out=g1_t[:, ib, :], in_=gp)

                # h2 = h1 @ W_self + Band @ g1 + b_gc
                h2_t = h2p.tile([128, 2, H], F32R, tag="h2")
                for mb in range(2):
                    hp = psB.tile([128, H], F32, tag="psb")
                    for jj in range(4):
                        nc.tensor.matmul(
                            out=hp, lhsT=_r(h1T_t[:, jj, ds(mb * 128, 128)]),
                            rhs=_r(wself[:, jj, :]),
                            start=(jj == 0), stop=False)
                    for ib in range(2):
                        nc.tensor.matmul(
                            out=hp, lhsT=_r(band0[:, ib, ds(mb * 128, 128)]),
                            rhs=_r(g1_t[:, ib, :]),
                            start=False, stop=False)
                    nc.tensor.matmul(out=hp, lhsT=ones1, rhs=bgc_row,
                                     start=False, stop=True)
                    cb.copy(out=h2_t[:, mb, :], in_=hp)
                    nc.scalar.dma_start(out=h2s_d[j, mb], in_=h2_t[:, mb, :])

                if DEBUG and j == 0 and "dbg_h2" in dbg:
                    nc.scalar.dma_start(out=dbg["dbg_h2"], in_=h2_t)

        # ------------------------------------------------------------------
        # Phase C: matching attention + classifier
        # ------------------------------------------------------------------
        with ExitStack() as P:
            wp = P.enter_context(tc.tile_pool(name="wC", bufs=1))
            wm = wp.tile([128, 12, DH], F32R)
            nc.sync.dma_start(
                out=wm,
                in_=wm_d.rearrange("(kt p) e -> p kt e", p=128).bitcast(F32R))
            wlin = wp.tile([128, 12, H], F32R)
            nc.sync.dma_start(
                out=wlin,
                in_=wlin_d.rearrange("(kt p) h -> p kt h", p=128).bitcast(F32R))
            wfc = wp.tile([128, 4, C], F32R)
            nc.sync.dma_start(
                out=wfc,
                in_=wfc_d.rearrange("(kt p) c -> p kt c", p=128).bitcast(F32R))

            xpool = P.enter_context(tc.tile_pool(name="xC", bufs=2))
            xTpool = P.enter_context(tc.tile_pool(name="xTC", bufs=2))
            h2pool = P.enter_context(tc.tile_pool(name="h2C", bufs=2))
            h2Tpool = P.enter_context(tc.tile_pool(name="h2TC", bufs=2))
            mtp = P.enter_context(tc.tile_pool(name="mtC", bufs=1))
            gsp = P.enter_context(tc.tile_pool(name="gsC", bufs=2))
            atp = P.enter_context(tc.tile_pool(name="atC", bufs=2))
            attp = P.enter_context(tc.tile_pool(name="attC", bufs=1))
            hidp = P.enter_context(tc.tile_pool(name="hidC", bufs=2))
            outp = P.enter_context(tc.tile_pool(name="outC", bufs=2))
            tiny = P.enter_context(tc.tile_pool(name="tinyC", bufs=4))
            scr = P.enter_context(tc.tile_pool(name="scrC", bufs=2))

            psT = P.enter_context(tc.tile_pool(name="psTC", bufs=2, space="PSUM"))
            psC = P.enter_context(tc.tile_pool(name="psC", bufs=4, space="PSUM"))

            for j in range(BPC):
                x_t, xT_t = _load_xT(nc, cb, xpool, xTpool, psT, feat, j, ident)
                h2_t = h2pool.tile([128, 2, H], F32R, tag="h2")
                for mb in range(2):
                    nc.sync.dma_start(out=h2_t[:, mb, :], in_=h2s_d[j, mb])
                h2T_t = h2Tpool.tile([128, 4, L], F32R, tag="h2T")
                for jj in range(4):
                    for mb in range(2):
                        tp = psT.tile([128, 128], F32R, tag="psT")
                        nc.tensor.transpose(
                            out=tp, in_=h2_t[:, mb, ds(jj * 128, 128)],
                            identity=ident)
                        cb.copy(out=h2T_t[:, jj, ds(mb * 128, 128)], in_=tp)

                def emT(kt):
                    return xT_t[:, kt, :] if kt < 8 else h2T_t[:, kt - 8, :]

                def em_lhsT(sb, dhb):
                    if dhb < 8:
                        return x_t[:, sb, ds(dhb * 128, 128)]
                    return h2_t[:, sb, ds((dhb - 8) * 128, 128)]

                # M_tT[e', t] = W_match.T @ emT  (+ b_match)
                mt_t = mtp.tile([128, 12, L], F32R, tag="mt")
                for eb in range(12):
                    mp = psC.tile([128, L], F32, tag="psc")
                    for kt in range(12):
                        nc.tensor.matmul(
                            out=mp, lhsT=_r(wm[:, kt, ds(eb * 128, 128)]),
                            rhs=_r(emT(kt)),
                            start=(kt == 0), stop=(kt == 11))
                    nc.vector.tensor_scalar(
                        out=mt_t[:, eb, :], in0=mp,
                        scalar1=bm_col[:, ds(eb, 1)], scalar2=None, op0=ALU.add)

                # G[t, s] = tanh(M_t @ em.T), softmax over s -> A
                a_t = gsp.tile([128, 2, L], F32R, tag="a")
                for tb in range(2):
                    gp = psC.tile([128, L], F32, tag="psc")
                    for kt in range(12):
                        nc.tensor.matmul(
                            out=gp, lhsT=_r(mt_t[:, kt, ds(tb * 128, 128)]),
                            rhs=_r(emT(kt)),
                            start=(kt == 0), stop=(kt == 11))
                    gs = scr.tile([128, L], F32, tag="gs")
                    nc.scalar.activation(gs, gp, AF.Tanh)
                    nm = tiny.tile([128, 1], F32, tag="nm")
                    nc.vector.tensor_reduce(out=nm, in_=gs, axis=AX.X,
                                            op=ALU.max, negate=True)
                    zz = tiny.tile([128, 1], F32, tag="zz")
                    nc.scalar.activation(a_t[:, tb, :], gs, AF.Exp, bias=nm,
                                         scale=1.0, accum_out=zz)
                    rz = tiny.tile([128, 1], F32, tag="rz")
                    nc.vector.reciprocal(rz, zz)
                    nc.vector.tensor_scalar(out=a_t[:, tb, :], in0=a_t[:, tb, :],
                                            scalar1=rz, scalar2=None, op0=ALU.mult)

                if DEBUG and j == 0 and "dbg_a" in dbg:
                    nc.scalar.dma_start(out=dbg["dbg_a"], in_=a_t)

                # AT[s, t]
                aT_t = atp.tile([128, 2, L], F32R, tag="aT")
                for tb in range(2):
                    for sb in range(2):
                        tp = psT.tile([128, 128], F32R, tag="psT")
                        nc.tensor.transpose(
                            out=tp, in_=a_t[:, tb, ds(sb * 128, 128)],
                            identity=ident)
                        cb.copy(out=aT_t[:, sb, ds(tb * 128, 128)], in_=tp)

                # attT[dh, t] = em.T @ A.T
                att_t = attp.tile([128, 12, L], F32R, tag="att")
                for dhb in range(12):
                    ap_ = psC.tile([128, L], F32, tag="psc")
                    for sb in range(2):
                        nc.tensor.matmul(
                            out=ap_, lhsT=_r(em_lhsT(sb, dhb)),
                            rhs=_r(aT_t[:, sb, :]),
                            start=(sb == 0), stop=(sb == 1))
                    cb.copy(out=att_t[:, dhb, :], in_=ap_)

                if DEBUG and j == 0 and "dbg_att" in dbg:
                    nc.scalar.dma_start(out=dbg["dbg_att"], in_=att_t)

                # hiddenT = relu(W_lin.T @ attT + b_lin)
                hid_t = hidp.tile([128, 4, L], F32R, tag="hid")
                for hb in range(4):
                    hp = psC.tile([128, L], F32, tag="psc")
                    for kt in range(12):
                        nc.tensor.matmul(
                            out=hp, lhsT=_r(wlin[:, kt, ds(hb * 128, 128)]),
                            rhs=_r(att_t[:, kt, :]),
                            start=(kt == 0), stop=(kt == 11))
                    nc.scalar.activation(hid_t[:, hb, :], hp, AF.Relu,
                                         bias=blin_col[:, ds(hb, 1)], scale=1.0)

                # logits + log_softmax
                out_t = outp.tile([128, 2, C], F32, tag="out")
                for tb in range(2):
                    lp = psC.tile([128, C], F32, tag="psc")
                    for hb in range(4):
                        nc.tensor.matmul(
                            out=lp, lhsT=_r(hid_t[:, hb, ds(tb * 128, 128)]),
                            rhs=_r(wfc[:, hb, :]),
                            start=(hb == 0), stop=False)
                    nc.tensor.matmul(out=lp, lhsT=_r(ones1), rhs=_r(bfc_row),
                                     start=False, stop=True)
                    nm = tiny.tile([128, 1], F32, tag="nm2")
                    nc.vector.tensor_reduce(out=nm, in_=lp, axis=AX.X,
                                            op=ALU.max, negate=True)
                    zc = scr.tile([128, C], F32, tag="zc")
                    nc.vector.tensor_scalar(out=zc, in0=lp, scalar1=nm,
                                            scalar2=None, op0=ALU.add)
                    ez = scr.tile([128, C], F32, tag="ez")
                    zz = tiny.tile([128, 1], F32, tag="zz2")
                    nc.scalar.activation(ez, zc, AF.Exp, accum_out=zz)
                    lnz = tiny.tile([128, 1], F32, tag="lnz")
                    nc.scalar.activation(lnz, zz, AF.Ln)
                    nc.vector.tensor_scalar(out=out_t[:, tb, :], in0=zc,
                                            scalar1=lnz, scalar2=None,
                                            op0=ALU.subtract)
                    nc.scalar.dma_start(out=out_d[ds(j * L + tb * 128, 128), :],
                                        in_=out_t[:, tb, :])


def _rgcn_half(nc, cb, tc, tiny, mskp, hallsb, psHall, psAgg, aggsb,
               u_all, spm_all, rd_all, btri, xT_t, wrel_half, j, a,
               aggp_d, store):
    """RGCN aggregation for speaker-of-source a (relations 4a..4a+3).

    Computes agg_c[k] = sum_i Msk_{a,d}[i,k] * (x[i] @ W_rel[4a+2c+d])
    into 4 PSUM tiles [c][mb].  If store, evacuate+DMA to aggp_d[j];
    else return the live psum tiles."""
    # per-(a, mb) row multipliers  m_a = sp_mask_a / d
    ma = tiny.tile([128, 2, 1], F32, tag="ma")
    for mb in range(2):
        nc.vector.tensor_tensor(out=ma[:, mb, :], in0=rd_all[:, j, mb, :],
                                in1=spm_all[:, j, a, mb, :], op=ALU.mult)

    agg_ps = [[psAgg.tile([128, H], F32, tag=f"agg{c}{mb}", name=f"agg_ps{c}{mb}")
               for mb in range(2)] for c in range(2)]

    for dbit in range(2):
        msk = mskp.tile([128, 2, L], F32R, tag="msk")
        for mb in range(2):
            nc.vector.tensor_tensor(out=msk[:, mb, :], in0=u_all[:, j, mb, :],
                                    in1=btri[:, dbit, mb, :], op=ALU.mult)
            nc.vector.tensor_scalar(out=msk[:, mb, :], in0=msk[:, mb, :],
                                    scalar1=ma[:, mb, :], scalar2=None,
                                    op0=ALU.mult)
        for c in range(2):
            ri = 2 * c + dbit
            hs = []
            for ib in range(2):
                hp = psHall.tile([128, H], F32, tag="hallp")
                for kt in range(8):
                    nc.tensor.matmul(
                        out=hp, lhsT=_r(xT_t[:, kt, ds(ib * 128, 128)]),
                        rhs=_r(wrel_half[:, ri, kt, :]),
                        start=(kt == 0), stop=(kt == 7))
                h = hallsb.tile([128, H], F32R, tag="hall")
                cb.copy(out=h, in_=hp)
                hs.append(h)
            for kh in range(2):
                for ib in range(2):
                    nc.tensor.matmul(
                        out=agg_ps[c][kh],
                        lhsT=_r(msk[:, ib, ds(kh * 128, 128)]),
                        rhs=_r(hs[ib]),
                        start=(dbit == 0 and ib == 0),
                        stop=(dbit == 1 and ib == 1))

    if store:
        for c in range(2):
            for mb in range(2):
                t = aggsb.tile([128, H], F32, tag="aggsb")
                cb.copy(out=t, in_=agg_ps[c][mb])
                nc.scalar.dma_start(out=aggp_d[j, c, mb], in_=t)
        return None
    return agg_ps


# ---------------------------------------------------------------------------
# Host-side runner (axon / PJRT, 8 cores)
# ---------------------------------------------------------------------------
@functools.cache
def _runner():
    import jax
    from concourse import bass2jax
    from concourse.bass2jax import _bass_exec_p
    from jax.experimental.shard_map import shard_map
    from jax.sharding import Mesh, PartitionSpec

    bass2jax.install_neuronx_cc_hook()
    nc = _build_nc()

    part_name = nc.partition_id_tensor.name if nc.partition_id_tensor else None
    in_names, out_names, out_avals, zero_shapes = [], [], [], []
    for alloc in nc.m.functions[0].allocations:
        if not isinstance(alloc, mybir.MemoryLocationSet):
            continue
        name = alloc.memorylocations[0].name
        if alloc.kind == "ExternalInput":
            if name != part_name:
                in_names.append(name)
        elif alloc.kind == "ExternalOutput":
            out_names.append(name)
            shape = tuple(alloc.tensor_shape)
            dtype = mybir.dt.np(alloc.dtype)
            out_avals.append(jax.core.ShapedArray(shape, dtype))
            zero_shapes.append((shape, dtype))
    n_params = len(in_names)
    all_in_names = in_names + out_names
    if part_name is not None:
        all_in_names = all_in_names + [part_name]

    def _bass_body(*args):
        operands = list(args)
        if part_name is not None:
            operands.append(bass2jax.partition_id_tensor())
        outs = _bass_exec_p.bind(
            *operands,
            out_avals=tuple(out_avals),
            in_names=tuple(all_in_names),
            out_names=tuple(out_names),
            lowering_input_output_aliases=(),
            sim_require_finite=True,
            sim_require_nnan=True,
            nc=nc,
        )
        return tuple(outs)

    devices = jax.devices()[:NCORES]
    mesh = Mesh(np.asarray(devices), ("core",))
    n_outs = len(out_names)
    sharded = jax.jit(
        shard_map(_bass_body, mesh=mesh,
                  in_specs=(PartitionSpec("core"),) * (n_params + n_outs),
                  out_specs=(PartitionSpec("core"),) * n_outs,
                  check_rep=False),
        donate_argnums=tuple(range(n_params, n_params + n_outs)),
        keep_unused=True,
    )

    def run(concat_inputs):
        zeros = [np.zeros((NCORES * s[0], *s[1:]), d) for s, d in zero_shapes]
        out_arrs = sharded(*concat_inputs, *zeros)
        return {name: np.asarray(out_arrs[i]) for i, name in enumerate(out_names)}

    return run, in_names, out_names


def _pack_inputs(inputs, in_names):
    """Build the concatenated (8*shape0, ...) arrays in in_names order."""
    feats = np.asarray(inputs["features"], np.float32)   # [L, B, D]
    spks = np.asarray(inputs["speakers"], np.int32)      # [L, B]
    per_name = {}
    per_name["features"] = np.concatenate(
        [np.ascontiguousarray(feats[:, c * BPC:(c + 1) * BPC, :])
         for c in range(NCORES)], axis=0)
    per_name["speakers"] = np.concatenate(
        [np.ascontiguousarray(spks[:, c * BPC:(c + 1) * BPC])
         for c in range(NCORES)], axis=0)
    for name in in_names:
        if name in ("features", "speakers"):
            continue
        arr = np.ascontiguousarray(np.asarray(inputs[name], np.float32))
        per_name[name] = np.concatenate([arr] * NCORES, axis=0)
    return [per_name[n] for n in in_names]


def kernel(**inputs):
    run, in_names, out_names = _runner()
    concat = _pack_inputs(inputs, in_names)
    outs = run(concat)
    full = outs["out"]  # [8 * BPC * L, C] already conversation-major
    return full.reshape(B * L, C).astype(np.float32)


if __name__ == "__main__":
    rng = np.random.default_rng(0)
    fake = {
        "features": rng.standard_normal((L, B, D), dtype=np.float32),
        "Wscalar": rng.standard_normal((D, L), dtype=np.float32) * 0.02,
        "W_rel": rng.standard_normal((R, D, H), dtype=np.float32) * 0.02,
        "W_root": rng.standard_normal((D, H), dtype=np.float32) * 0.02,
        "b_rgcn": np.zeros(H, np.float32),
        "W_nbr": rng.standard_normal((H, H), dtype=np.float32) * 0.02,
        "W_self": rng.standard_normal((H, H), dtype=np.float32) * 0.02,
        "b_gc": np.zeros(H, np.float32),
        "W_match": rng.standard_normal((DH, DH), dtype=np.float32) * 0.02,
        "b_match": np.zeros(DH, np.float32),
        "W_lin": rng.standard_normal((DH, H), dtype=np.float32) * 0.02,
        "b_lin": np.zeros(H, np.float32),
        "W_fc": rng.standard_normal((H, C), dtype=np.float32) * 0.02,
        "b_fc": np.zeros(C, np.float32),
        "speakers": rng.integers(0, 2, (L, B)).astype(np.int32),
        "pair_i": np.zeros(1, np.int32),
        "pair_k": np.zeros(1, np.int32),
    }
    out = kernel(**fake)
    print("kernel output", out.shape, out.dtype, float(np.abs(out).max()))


# revision 15
# speedup vs baseline: 1.0958x; 1.0958x over previous
"""Trainium2 Bass kernel for nn_DialogueGNNModel (gnn_message_passing).

Data-parallel over conversations: 64 conversations split 8-per-core across
8 NeuronCores. Per conversation (L=256 nodes, D=1024 features):

  1. edge attention:  scaleT[m,s] = (Wscalar.T @ x.T);  softmax over s;
     window-renormalized scores (band |m-s|<=10, off-band weight 1e-10).
  2. RGCN over the banded window graph, relations r = 4*sp[i]+2*sp[k]+dir:
     done as dense masked [L,L]@[L,H] matmuls per relation
     (mask = scores * band * dir-triangle * speaker masks).
  3. GraphConv:  h2 = Band@ (h1@W_nbr) + h1@W_self + b_gc.
  4. MatchingAttention over em=[x|h2] and classifier -> log_softmax.

All big matmuls run as float32r (FP22 single-pass, full PE rate at N>=256).
The kernel is staged in three weight-resident phases (A1: relations 0-3,
A2: relations 4-7 + root + GraphConv, C: attention + classifier) so SBUF
holds each phase's weights once while all 8 conversations stream through.
"""

import functools
from contextlib import ExitStack

import numpy as np

import concourse.bass as bass
import concourse.mybir as mybir
import concourse.tile as tile
from concourse.bass import ds
from concourse.masks import make_identity
from concourse.vector_clock import ScopedClock

L, B, D, H, R, C, WIN = 256, 64, 1024, 512, 8, 6, 10
DH = D + H
NCORES = 8
BPC = B // NCORES  # conversations per core

dt = mybir.dt
F32 = dt.float32
F32R = dt.float32r
I32 = dt.int32
AF = mybir.ActivationFunctionType
ALU = mybir.AluOpType
AX = mybir.AxisListType

DEBUG = False  # extra dram dumps of conv-0 intermediates


# ---------------------------------------------------------------------------
# Workaround: this walrus build rejects >1 sem-wait on a CTRL (Drain/Nop)
# instruction ("Too many sync wait commands").  Tile's kernel-tail drain puts
# the whole global-clock wait set on one Drain; split the waits across a
# chain of single-wait sync-engine NOPs instead.
# ---------------------------------------------------------------------------
def _drain_and_barrier_split(self, tick_clock, wait_clock):
    nc = self.nc
    probe = nc.sync.nop(nofuse=True)
    wait_clock.add_sem_waits(probe.ins, ScopedClock({None: tick_clock.global_clock}))
    si = probe.ins.sync_info
    waits = list(si.on_wait or []) if si is not None else []
    if len(waits) > 1:
        si.on_wait = waits[:1]
        for w in waits[1:]:
            n = nc.sync.nop(nofuse=True)
            n.ins.sync_info = mybir.SyncInfo(on_wait=[w], on_update=[])
    nc.sync.drain()

    nc.all_engine_barrier()
    assert self.sems is not None
    popped = nc._tile_sem_poison_stack.pop()
    assert popped is self._sem_poison
    nc.clear_and_free_semaphores(list(self.sems.allocated().values()))
    nc.all_engine_barrier()


_orig_lower_ordered = tile.TileContext._lower_ordered_insts


def _lower_ordered_split_waits(self, ordered):
    """This walrus build accepts at most ONE sem-wait per instruction.

    Tile's sem-assignment attaches every required cross-engine wait to the
    consuming instruction.  Move all but the last wait onto same-engine NOPs
    inserted just before it — the engine then blocks on each wait in turn
    before executing the instruction, which is semantically identical."""
    nc = self.nc
    for insts in ordered.values():
        out = []
        for inst in insts:
            si = inst.sync_info
            if si is not None and si.on_wait and len(si.on_wait) > 1:
                waits = list(si.on_wait)
                si.on_wait = [waits[-1]]
                for w in waits[:-1]:
                    nop = mybir.InstNoOp(
                        name=f"I-{nc.next_id()}", ins=[], outs=[],
                        engine=inst.engine, bass_nofuse=True)
                    nop.sync_info = mybir.SyncInfo(on_wait=[w], on_update=[])
                    out.append(nop)
            out.append(inst)
        insts[:] = out
    return _orig_lower_ordered(self, ordered)


def _install_patches():
    tile.TileContext._drain_and_barrier = _drain_and_barrier_split
    tile.TileContext._lower_ordered_insts = _lower_ordered_split_waits


def _r(ap):
    """Reinterpret an fp32 AP as float32r for full-rate PE matmuls."""
    return ap.bitcast(F32R)


class _CopyBalancer:
    """Alternate PSUM->SBUF evacuation copies between DVE and ACT."""

    def __init__(self, nc):
        self.nc = nc
        self.i = 0

    def copy(self, out, in_):
        if self.i % 2 == 0:
            self.nc.vector.tensor_copy(out=out, in_=in_)
        else:
            self.nc.scalar.copy(out=out, in_=in_)
        self.i += 1


def _build_nc():
    _install_patches()
    nc = bass.Bass("TRN2", target_bir_lowering=False, debug=False,
                   num_devices=NCORES)

    feat = nc.dram_tensor("features", [L, BPC, D], F32, kind="ExternalInput").ap()
    spk = nc.dram_tensor("speakers", [L, BPC], I32, kind="ExternalInput").ap()
    wsc_d = nc.dram_tensor("Wscalar", [D, L], F32, kind="ExternalInput").ap()
    wrel_d = nc.dram_tensor("W_rel", [R, D, H], F32, kind="ExternalInput").ap()
    wroot_d = nc.dram_tensor("W_root", [D, H], F32, kind="ExternalInput").ap()
    brg_d = nc.dram_tensor("b_rgcn", [H], F32, kind="ExternalInput").ap()
    wnbr_d = nc.dram_tensor("W_nbr", [H, H], F32, kind="ExternalInput").ap()
    wself_d = nc.dram_tensor("W_self", [H, H], F32, kind="ExternalInput").ap()
    bgc_d = nc.dram_tensor("b_gc", [H], F32, kind="ExternalInput").ap()
    wm_d = nc.dram_tensor("W_match", [DH, DH], F32, kind="ExternalInput").ap()
    bm_d = nc.dram_tensor("b_match", [DH], F32, kind="ExternalInput").ap()
    wlin_d = nc.dram_tensor("W_lin", [DH, H], F32, kind="ExternalInput").ap()
    blin_d = nc.dram_tensor("b_lin", [H], F32, kind="ExternalInput").ap()
    wfc_d = nc.dram_tensor("W_fc", [H, C], F32, kind="ExternalInput").ap()
    bfc_d = nc.dram_tensor("b_fc", [C], F32, kind="ExternalInput").ap()

    out_d = nc.dram_tensor("out", [BPC * L, C], F32, kind="ExternalOutput").ap()

    aggp_d = nc.dram_tensor("aggp_scratch", [BPC, 2, 2, 128, H], F32).ap()
    xts_d = nc.dram_tensor("xT_scratch", [BPC, 128, 8, L], F32R).ap()
    h1s_d = nc.dram_tensor("h1_scratch", [BPC, 2, 128, H], F32R).ap()
    h2s_d = nc.dram_tensor("h2_scratch", [BPC, 2, 128, H], F32R).ap()

    dbg = {}
    if DEBUG:
        for name, shape in [
            ("dbg_u", [128, 2, L]), ("dbg_d", [128, 2, 1]),
            ("dbg_h1", [128, 2, H]), ("dbg_h2", [128, 2, H]),
            ("dbg_att", [128, 12, L]), ("dbg_a", [128, 2, L]),
        ]:
            dbg[name] = nc.dram_tensor(name, shape, F32, kind="ExternalOutput").ap()

    with tile.TileContext(nc) as tc:
        _body(tc, feat, spk, wsc_d, wrel_d, wroot_d, brg_d, wnbr_d, wself_d,
              bgc_d, wm_d, bm_d, wlin_d, blin_d, wfc_d, bfc_d, out_d,
              aggp_d, xts_d, h1s_d, h2s_d, dbg)
    return nc


def _load_xT(nc, cb, xpool, xTpool, psT, feat, xts_d, j, ident):
    """DMA x (conv j), produce its transpose xT via PE transposes, and
    spill xT to DRAM scratch for later phases."""
    x_t = xpool.tile([128, 2, D], F32R, tag="x")
    for mb in range(2):
        nc.scalar.dma_start(out=x_t[:, mb, :],
                            in_=feat[ds(mb * 128, 128), j, :].bitcast(F32R))
    xT_t = xTpool.tile([128, 8, L], F32R, tag="xT")
    for kt in range(8):
        for mb in range(2):
            tp = psT.tile([128, 128], F32R, tag="psT")
            nc.tensor.transpose(out=tp, in_=x_t[:, mb, ds(kt * 128, 128)],
                                identity=ident)
            cb.copy(out=xT_t[:, kt, ds(mb * 128, 128)], in_=tp)
    nc.scalar.dma_start(out=xts_d[j], in_=xT_t)
    return x_t, xT_t


def _body(tc, feat, spk, wsc_d, wrel_d, wroot_d, brg_d, wnbr_d, wself_d,
          bgc_d, wm_d, bm_d, wlin_d, blin_d, wfc_d, bfc_d, out_d,
          aggp_d, xts_d, h1s_d, h2s_d, dbg):
    nc = tc.nc
    cb = _CopyBalancer(nc)

    with ExitStack() as S:
        consts = S.enter_context(tc.tile_pool(name="consts", bufs=1))

        identf = consts.tile([128, 128], F32)
        make_identity(nc, identf)
        ident = consts.tile([128, 128], F32R)
        nc.vector.tensor_copy(out=ident, in_=identf)

        # Window masks. Layout [p, mb, s]: row = node i = mb*128+p, col = s.
        winm = consts.tile([128, 2, L], F32)   # 1.0 in-band else 1e-10
        band0f = consts.tile([128, 2, L], F32)  # 1.0 in-band else 0.0
        band0 = consts.tile([128, 2, L], F32R)
        btri = consts.tile([128, 2, 2, L], F32)  # [p, dbit, mb, s]
        nc.gpsimd.memset(winm, 1.0)
        nc.gpsimd.memset(band0f, 1.0)
        for mb in range(2):
            for fill, t in ((1e-10, winm), (0.0, band0f)):
                nc.gpsimd.affine_select(
                    out=t[:, mb, :], in_=t[:, mb, :], compare_op=ALU.is_ge,
                    fill=fill, base=mb * 128 + WIN, channel_multiplier=1,
                    pattern=[[-1, L]])
                nc.gpsimd.affine_select(
                    out=t[:, mb, :], in_=t[:, mb, :], compare_op=ALU.is_ge,
                    fill=fill, base=WIN - mb * 128, channel_multiplier=-1,
                    pattern=[[1, L]])
            # dbit=0: i < s ;  dbit=1: i >= s
            nc.vector.tensor_copy(out=btri[:, 0, mb, :], in_=band0f[:, mb, :])
            nc.gpsimd.affine_select(
                out=btri[:, 0, mb, :], in_=btri[:, 0, mb, :],
                compare_op=ALU.is_gt, fill=0.0, base=-mb * 128,
                channel_multiplier=-1, pattern=[[1, L]])
            nc.vector.tensor_copy(out=btri[:, 1, mb, :], in_=band0f[:, mb, :])
            nc.gpsimd.affine_select(
                out=btri[:, 1, mb, :], in_=btri[:, 1, mb, :],
                compare_op=ALU.is_ge, fill=0.0, base=mb * 128,
                channel_multiplier=1, pattern=[[-1, L]])
        nc.vector.tensor_copy(out=band0, in_=band0f)

        # biases
        brg_row = consts.tile([1, H], F32R)
        bgc_row = consts.tile([1, H], F32R)
        bfc_row = consts.tile([1, C], F32R)
        ones1f = consts.tile([1, 128], F32)
        ones1 = consts.tile([1, 128], F32R)
        bm_col = consts.tile([128, 12], F32)
        blin_col = consts.tile([128, 4], F32)
        nc.gpsimd.memset(ones1f, 1.0)
        nc.vector.tensor_copy(out=ones1, in_=ones1f)
        nc.sync.dma_start(out=brg_row, in_=brg_d.unsqueeze(0).bitcast(F32R))
        nc.sync.dma_start(out=bgc_row, in_=bgc_d.unsqueeze(0).bitcast(F32R))
        nc.sync.dma_start(out=bfc_row, in_=bfc_d.unsqueeze(0).bitcast(F32R))
        nc.sync.dma_start(out=bm_col, in_=bm_d.rearrange("(blk p) -> p blk", p=128))
        nc.sync.dma_start(out=blin_col, in_=blin_d.rearrange("(blk p) -> p blk", p=128))

        # partition-replicated b_rgcn / b_gc (ones-column outer product)
        brg_rep = consts.tile([128, H], F32)
        bgc_rep = consts.tile([128, H], F32)
        with tc.tile_pool(name="psSetup", bufs=2, space="PSUM") as psS:
            for row, rep in ((brg_row, brg_rep), (bgc_row, bgc_rep)):
                bp = psS.tile([128, H], F32, tag="pss")
                nc.tensor.matmul(out=bp, lhsT=ones1, rhs=row, start=True,
                                 stop=True)
                nc.vector.tensor_copy(out=rep, in_=bp)

        # speakers, int32 [L, BPC] -> [128, mb, BPC]
        spk_i = consts.tile([128, 2, BPC], I32)
        for mb in range(2):
            nc.sync.dma_start(out=spk_i[:, mb, :], in_=spk[ds(mb * 128, 128), :])

        with ExitStack() as SP:
            pers = SP.enter_context(tc.tile_pool(name="pers", bufs=1))
            # persistent per-conversation state (A1 -> A2)
            u_all = pers.tile([128, BPC, 2, L], F32)      # exp(scale - max)
            spm_all = pers.tile([128, BPC, 2, 2, 1], F32)  # [p, j, a, mb, 1]
            rd_all = pers.tile([128, BPC, 2, 1], F32)      # 1/denominator

            # --------------------------------------------------------------
            # Phase A1: scale/softmax/scores + RGCN relations 0-3 (a=0)
            # --------------------------------------------------------------
            with ExitStack() as P:
                wp = P.enter_context(tc.tile_pool(name="wA1", bufs=1))
                wsc = wp.tile([128, 8, L], F32R)
                nc.sync.dma_start(
                    out=wsc,
                    in_=wsc_d.rearrange("(kt p) m -> p kt m", p=128).bitcast(F32R))
                wrelA = wp.tile([128, 4, 8, H], F32R)
                for ri in range(4):
                    nc.sync.dma_start(
                        out=wrelA[:, ri],
                        in_=wrel_d[ri].rearrange("(kt p) h -> p kt h", p=128).bitcast(F32R))

                xpool = P.enter_context(tc.tile_pool(name="xA1", bufs=2))
                xTpool = P.enter_context(tc.tile_pool(name="xTA1", bufs=2))
                tiny = P.enter_context(tc.tile_pool(name="tinyA1", bufs=4))
                mskp = P.enter_context(tc.tile_pool(name="mskA1", bufs=2))
                hallsb = P.enter_context(tc.tile_pool(name="hallsbA1", bufs=4))
                aggsb = P.enter_context(tc.tile_pool(name="aggsbA1", bufs=2))
                scr = P.enter_context(tc.tile_pool(name="scrA1", bufs=2))

                psT = P.enter_context(tc.tile_pool(name="psTA1", bufs=2, space="PSUM"))
                psHall = P.enter_context(tc.tile_pool(name="psHallA1", bufs=2, space="PSUM"))
                psAgg = P.enter_context(tc.tile_pool(name="psAggA1", bufs=1, space="PSUM"))

                for j in range(BPC):
                    x_t, xT_t = _load_xT(nc, cb, xpool, xTpool, psT, feat,
                                         xts_d, j, ident)

                    # --- speaker masks for conv j ---
                    for mb in range(2):
                        spf = tiny.tile([128, 1], F32, tag="spf")
                        nc.vector.tensor_copy(out=spf, in_=spk_i[:, mb, ds(j, 1)])
                        nc.vector.tensor_copy(out=spm_all[:, j, 1, mb, :], in_=spf)
                        nc.vector.tensor_scalar(
                            out=spm_all[:, j, 0, mb, :], in0=spf, scalar1=-1.0,
                            scalar2=1.0, op0=ALU.mult, op1=ALU.add)

                    # --- scale + softmax-over-s + window denominator ---
                    for mb in range(2):
                        sc = psHall.tile([128, L], F32, tag="hallp")
                        for kt in range(8):
                            nc.tensor.matmul(
                                out=sc, lhsT=_r(wsc[:, kt, ds(mb * 128, 128)]),
                                rhs=_r(xT_t[:, kt, :]),
                                start=(kt == 0), stop=(kt == 7))
                        nm = tiny.tile([128, 1], F32, tag="nm")
                        nc.vector.tensor_reduce(out=nm, in_=sc, axis=AX.X,
                                                op=ALU.max, negate=True)
                        u_ap = u_all[:, j, mb, :]
                        nc.scalar.activation(u_ap, sc, AF.Exp, bias=nm, scale=1.0)
                        tmp = scr.tile([128, L], F32, tag="scr")
                        nc.vector.tensor_tensor(out=tmp, in0=u_ap,
                                                in1=winm[:, mb, :], op=ALU.mult)
                        dd = tiny.tile([128, 1], F32, tag="dd")
                        nc.vector.tensor_reduce(out=dd, in_=tmp, axis=AX.X,
                                                op=ALU.add)
                        nc.vector.reciprocal(rd_all[:, j, mb, :], dd)

                    _rgcn_half(nc, cb, tc, tiny, mskp, hallsb, psHall, psAgg,
                               aggsb, u_all, spm_all, rd_all, btri, xT_t, wrelA,
                               j, a=0, aggp_d=aggp_d, store=True)

            # --------------------------------------------------------------
            # Phase A2: relations 4-7 (a=1) + W_root -> h1 to scratch
            # --------------------------------------------------------------
            with ExitStack() as P:
                wp = P.enter_context(tc.tile_pool(name="wA2", bufs=1))
                wroot = wp.tile([128, 8, H], F32R)
                nc.sync.dma_start(
                    out=wroot,
                    in_=wroot_d.rearrange("(kt p) h -> p kt h", p=128).bitcast(F32R))
                wrelB = wp.tile([128, 4, 8, H], F32R)
                for ri in range(4):
                    nc.sync.dma_start(
                        out=wrelB[:, ri],
                        in_=wrel_d[4 + ri].rearrange("(kt p) h -> p kt h", p=128).bitcast(F32R))

                xTpool = P.enter_context(tc.tile_pool(name="xTA2", bufs=2))
                tiny = P.enter_context(tc.tile_pool(name="tinyA2", bufs=4))
                mskp = P.enter_context(tc.tile_pool(name="mskA2", bufs=2))
                hallsb = P.enter_context(tc.tile_pool(name="hallsbA2", bufs=4))
                rootsb = P.enter_context(tc.tile_pool(name="rootA2", bufs=1))
                apsb = P.enter_context(tc.tile_pool(name="apsbA2", bufs=2))
                h1p = P.enter_context(tc.tile_pool(name="h1A2", bufs=2))
                scr = P.enter_context(tc.tile_pool(name="scrA2", bufs=2))

                psHall = P.enter_context(tc.tile_pool(name="psHallA2", bufs=2, space="PSUM"))
                psAgg = P.enter_context(tc.tile_pool(name="psAggA2", bufs=1, space="PSUM"))

                for j in range(BPC):
                    xT_t = xTpool.tile([128, 8, L], F32R, tag="xT")
                    nc.scalar.dma_start(out=xT_t, in_=xts_d[j])

                    # partial sums from A1 (prefetch early)
                    ap_t = apsb.tile([128, 2, 2, H], F32, tag="apsb")
                    for c in range(2):
                        for mb in range(2):
                            nc.scalar.dma_start(out=ap_t[:, c, mb, :],
                                                in_=aggp_d[j, c, mb])

                    # root transform x @ W_root + b_rgcn
                    root_t = rootsb.tile([128, 2, H], F32, tag="root")
                    for mb in range(2):
                        rp = psHall.tile([128, H], F32, tag="hallp")
                        for kt in range(8):
                            nc.tensor.matmul(
                                out=rp, lhsT=_r(xT_t[:, kt, ds(mb * 128, 128)]),
                                rhs=_r(wroot[:, kt, :]),
                                start=(kt == 0), stop=(kt == 7))
                        nc.vector.tensor_tensor(out=root_t[:, mb, :], in0=rp,
                                                in1=brg_rep, op=ALU.add)

                    agg_ps = _rgcn_half(nc, cb, tc, tiny, mskp, hallsb, psHall,
                                        psAgg, None, u_all, spm_all, rd_all,
                                        btri, xT_t, wrelB, j, a=1, aggp_d=None,
                                        store=False)

                    # h1 = sel(agg) + root (incl b_rgcn)  -> h1 scratch
                    h1_t = h1p.tile([128, 2, H], F32R, tag="h1")
                    for mb in range(2):
                        t0 = scr.tile([128, H], F32, tag="t0")
                        t1 = scr.tile([128, H], F32, tag="t1")
                        nc.vector.tensor_tensor(out=t0, in0=agg_ps[0][mb],
                                                in1=ap_t[:, 0, mb, :], op=ALU.add)
                        nc.vector.tensor_tensor(out=t1, in0=agg_ps[1][mb],
                                                in1=ap_t[:, 1, mb, :], op=ALU.add)
                        nc.vector.tensor_scalar(out=t0, in0=t0,
                                                scalar1=spm_all[:, j, 0, mb, :],
                                                scalar2=None, op0=ALU.mult)
                        nc.vector.tensor_scalar(out=t1, in0=t1,
                                                scalar1=spm_all[:, j, 1, mb, :],
                                                scalar2=None, op0=ALU.mult)
                        nc.vector.tensor_tensor(out=h1_t[:, mb, :], in0=t0,
                                                in1=t1, op=ALU.add)
                        nc.vector.tensor_tensor(out=h1_t[:, mb, :],
                                                in0=h1_t[:, mb, :],
                                                in1=root_t[:, mb, :], op=ALU.add)
                        nc.scalar.dma_start(out=h1s_d[j, mb], in_=h1_t[:, mb, :])

        # ------------------------------------------------------------------
        # Phases B + C with C-weight prefetch during B
        # ------------------------------------------------------------------
        with ExitStack() as SB:
            wpc = SB.enter_context(tc.tile_pool(name="wC", bufs=1))
            wm = wpc.tile([128, 12, DH], F32R)
            nc.sync.dma_start(
                out=wm,
                in_=wm_d.rearrange("(kt p) e -> p kt e", p=128).bitcast(F32R))
            wlin = wpc.tile([128, 12, H], F32R)
            nc.sync.dma_start(
                out=wlin,
                in_=wlin_d.rearrange("(kt p) h -> p kt h", p=128).bitcast(F32R))
            wfc = wpc.tile([128, 4, C], F32R)
            nc.sync.dma_start(
                out=wfc,
                in_=wfc_d.rearrange("(kt p) c -> p kt c", p=128).bitcast(F32R))

            # ---- Phase B: GraphConv ----
            with ExitStack() as P:
                wp = P.enter_context(tc.tile_pool(name="wB", bufs=1))
                wnbr = wp.tile([128, 4, H], F32R)
                nc.sync.dma_start(
                    out=wnbr,
                    in_=wnbr_d.rearrange("(kt p) h -> p kt h", p=128).bitcast(F32R))
                wself = wp.tile([128, 4, H], F32R)
                nc.sync.dma_start(
                    out=wself,
                    in_=wself_d.rearrange("(kt p) h -> p kt h", p=128).bitcast(F32R))

                h1pool = P.enter_context(tc.tile_pool(name="h1B", bufs=2))
                h1Tp = P.enter_context(tc.tile_pool(name="h1TB", bufs=2))
                g1p = P.enter_context(tc.tile_pool(name="g1B", bufs=2))
                h2p = P.enter_context(tc.tile_pool(name="h2B", bufs=2))

                psT = P.enter_context(tc.tile_pool(name="psTB", bufs=2, space="PSUM"))
                psB = P.enter_context(tc.tile_pool(name="psB", bufs=2, space="PSUM"))

                for j in range(BPC):
                    h1_t = h1pool.tile([128, 2, H], F32R, tag="h1")
                    for mb in range(2):
                        nc.scalar.dma_start(out=h1_t[:, mb, :], in_=h1s_d[j, mb])

                    # h1T [128, jj, 256]
                    h1T_t = h1Tp.tile([128, 4, L], F32R, tag="h1T")
                    for jj in range(4):
                        for mb in range(2):
                            tp = psT.tile([128, 128], F32R, tag="psT")
                            nc.tensor.transpose(
                                out=tp, in_=h1_t[:, mb, ds(jj * 128, 128)],
                                identity=ident)
                            cb.copy(out=h1T_t[:, jj, ds(mb * 128, 128)], in_=tp)

                    # g1 = h1 @ W_nbr
                    g1_t = g1p.tile([128, 2, H], F32R, tag="g1")
                    for ib in range(2):
                        gp = psB.tile([128, H], F32, tag="psb")
                        for jj in range(4):
                            nc.tensor.matmul(
                                out=gp, lhsT=_r(h1T_t[:, jj, ds(ib * 128, 128)]),
                                rhs=_r(wnbr[:, jj, :]),
                                start=(jj == 0), stop=(jj == 3))
                        cb.copy(out=g1_t[:, ib, :], in_=gp)

                    # h2 = h1 @ W_self + Band @ g1 + b_gc
                    h2_t = h2p.tile([128, 2, H], F32R, tag="h2")
                    for mb in range(2):
                        hp = psB.tile([128, H], F32, tag="psb")
                        for jj in range(4):
                            nc.tensor.matmul(
                                out=hp, lhsT=_r(h1T_t[:, jj, ds(mb * 128, 128)]),
                                rhs=_r(wself[:, jj, :]),
                                start=(jj == 0), stop=False)
                        for ib in range(2):
                            nc.tensor.matmul(
                                out=hp, lhsT=_r(band0[:, ib, ds(mb * 128, 128)]),
                                rhs=_r(g1_t[:, ib, :]),
                                start=False, stop=(ib == 1))
                        nc.vector.tensor_tensor(out=h2_t[:, mb, :], in0=hp,
                                                in1=bgc_rep, op=ALU.add)
                        nc.scalar.dma_start(out=h2s_d[j, mb], in_=h2_t[:, mb, :])

            # ---- Phase C: matching attention + classifier ----
            with ExitStack() as P:
                xpool = P.enter_context(tc.tile_pool(name="xC", bufs=2))
                xTpool = P.enter_context(tc.tile_pool(name="xTC", bufs=2))
                h2pool = P.enter_context(tc.tile_pool(name="h2C", bufs=2))
                h2Tpool = P.enter_context(tc.tile_pool(name="h2TC", bufs=2))
                mtp = P.enter_context(tc.tile_pool(name="mtC", bufs=1))
                gsp = P.enter_context(tc.tile_pool(name="gsC", bufs=2))
                atp = P.enter_context(tc.tile_pool(name="atC", bufs=2))
                attp = P.enter_context(tc.tile_pool(name="attC", bufs=1))
                hidp = P.enter_context(tc.tile_pool(name="hidC", bufs=2))
                outp = P.enter_context(tc.tile_pool(name="outC", bufs=2))
                tiny = P.enter_context(tc.tile_pool(name="tinyC", bufs=4))
                scr = P.enter_context(tc.tile_pool(name="scrC", bufs=2))

                psT = P.enter_context(tc.tile_pool(name="psTC", bufs=2, space="PSUM"))
                psC = P.enter_context(tc.tile_pool(name="psC", bufs=4, space="PSUM"))

                for j in range(BPC):
                    x_t = xpool.tile([128, 2, D], F32R, tag="x")
                    for mb in range(2):
                        nc.scalar.dma_start(
                            out=x_t[:, mb, :],
                            in_=feat[ds(mb * 128, 128), j, :].bitcast(F32R))
                    xT_t = xTpool.tile([128, 8, L], F32R, tag="xT")
                    nc.scalar.dma_start(out=xT_t, in_=xts_d[j])
                    h2_t = h2pool.tile([128, 2, H], F32R, tag="h2")
                    for mb in range(2):
                        nc.scalar.dma_start(out=h2_t[:, mb, :], in_=h2s_d[j, mb])
                    h2T_t = h2Tpool.tile([128, 4, L], F32R, tag="h2T")
                    for jj in range(4):
                        for mb in range(2):
                            tp = psT.tile([128, 128], F32R, tag="psT")
                            nc.tensor.transpose(
                                out=tp, in_=h2_t[:, mb, ds(jj * 128, 128)],
                                identity=ident)
                            cb.copy(out=h2T_t[:, jj, ds(mb * 128, 128)], in_=tp)

                    def emT(kt):
                        return xT_t[:, kt, :] if kt < 8 else h2T_t[:, kt - 8, :]

                    def em_lhsT(sb, dhb):
                        if dhb < 8:
                            return x_t[:, sb, ds(dhb * 128, 128)]
                        return h2_t[:, sb, ds((dhb - 8) * 128, 128)]

                    # M_tT[e', t] = W_match.T @ emT  (+ b_match)
                    mt_t = mtp.tile([128, 12, L], F32R, tag="mt")
                    for eb in range(12):
                        mp = psC.tile([128, L], F32, tag="psc")
                        for kt in range(12):
                            nc.tensor.matmul(
                                out=mp, lhsT=_r(wm[:, kt, ds(eb * 128, 128)]),
                                rhs=_r(emT(kt)),
                                start=(kt == 0), stop=(kt == 11))
                        nc.vector.tensor_scalar(
                            out=mt_t[:, eb, :], in0=mp,
                            scalar1=bm_col[:, ds(eb, 1)], scalar2=None, op0=ALU.add)

                    # G[t, s] = tanh(M_t @ em.T), softmax over s -> A
                    a_t = gsp.tile([128, 2, L], F32R, tag="a")
                    for tb in range(2):
                        gp = psC.tile([128, L], F32, tag="psc")
                        for kt in range(12):
                            nc.tensor.matmul(
                                out=gp, lhsT=_r(mt_t[:, kt, ds(tb * 128, 128)]),
                                rhs=_r(emT(kt)),
                                start=(kt == 0), stop=(kt == 11))
                        gs = scr.tile([128, L], F32, tag="gs")
                        nc.scalar.activation(gs, gp, AF.Tanh)
                        nm = tiny.tile([128, 1], F32, tag="nm")
                        nc.vector.tensor_reduce(out=nm, in_=gs, axis=AX.X,
                                                op=ALU.max, negate=True)
                        zz = tiny.tile([128, 1], F32, tag="zz")
                        nc.scalar.activation(a_t[:, tb, :], gs, AF.Exp, bias=nm,
                                             scale=1.0, accum_out=zz)
                        rz = tiny.tile([128, 1], F32, tag="rz")
                        nc.vector.reciprocal(rz, zz)
                        nc.vector.tensor_scalar(out=a_t[:, tb, :],
                                                in0=a_t[:, tb, :],
                                                scalar1=rz, scalar2=None,
                                                op0=ALU.mult)

                    # AT[s, t]
                    aT_t = atp.tile([128, 2, L], F32R, tag="aT")
                    for tb in range(2):
                        for sb in range(2):
                            tp = psT.tile([128, 128], F32R, tag="psT")
                            nc.tensor.transpose(
                                out=tp, in_=a_t[:, tb, ds(sb * 128, 128)],
                                identity=ident)
                            cb.copy(out=aT_t[:, sb, ds(tb * 128, 128)], in_=tp)

                    # attT[dh, t] = em.T @ A.T
                    att_t = attp.tile([128, 12, L], F32R, tag="att")
                    for dhb in range(12):
                        ap_ = psC.tile([128, L], F32, tag="psc")
                        for sb in range(2):
                            nc.tensor.matmul(
                                out=ap_, lhsT=_r(em_lhsT(sb, dhb)),
                                rhs=_r(aT_t[:, sb, :]),
                                start=(sb == 0), stop=(sb == 1))
                        cb.copy(out=att_t[:, dhb, :], in_=ap_)

                    # hiddenT = relu(W_lin.T @ attT + b_lin)
                    hid_t = hidp.tile([128, 4, L], F32R, tag="hid")
                    for hb in range(4):
                        hp = psC.tile([128, L], F32, tag="psc")
                        for kt in range(12):
                            nc.tensor.matmul(
                                out=hp, lhsT=_r(wlin[:, kt, ds(hb * 128, 128)]),
                                rhs=_r(att_t[:, kt, :]),
                                start=(kt == 0), stop=(kt == 11))
                        nc.scalar.activation(hid_t[:, hb, :], hp, AF.Relu,
                                             bias=blin_col[:, ds(hb, 1)], scale=1.0)

                    # logits + log_softmax
                    out_t = outp.tile([128, 2, C], F32, tag="out")
                    for tb in range(2):
                        lp = psC.tile([128, C], F32, tag="psc")
                        for hb in range(4):
                            nc.tensor.matmul(
                                out=lp, lhsT=_r(hid_t[:, hb, ds(tb * 128, 128)]),
                                rhs=_r(wfc[:, hb, :]),
                                start=(hb == 0), stop=False)
                        nc.tensor.matmul(out=lp, lhsT=ones1, rhs=bfc_row,
                                         start=False, stop=True)
                        nm = tiny.tile([128, 1], F32, tag="nm2")
                        nc.vector.tensor_reduce(out=nm, in_=lp, axis=AX.X,
                                                op=ALU.max, negate=True)
                        zc = scr.tile([128, C], F32, tag="zc")
                        nc.vector.tensor_scalar(out=zc, in0=lp, scalar1=nm,
                                                scalar2=None, op0=ALU.add)
                        ez = scr.tile([128, C], F32, tag="ez")
                        zz = tiny.tile([128, 1], F32, tag="zz2")
                        nc.scalar.activation(ez, zc, AF.Exp, accum_out=zz)
                        lnz = tiny.tile([128, 1], F32, tag="lnz")
                        nc.scalar.activation(lnz, zz, AF.Ln)
                        nc.vector.tensor_scalar(out=out_t[:, tb, :], in0=zc,
                                                scalar1=lnz, scalar2=None,
                                                op0=ALU.subtract)
                        nc.scalar.dma_start(out=out_d[ds(j * L + tb * 128, 128), :],
                                            in_=out_t[:, tb, :])


def _rgcn_half(nc, cb, tc, tiny, mskp, hallsb, psHall, psAgg, aggsb,
               u_all, spm_all, rd_all, btri, xT_t, wrel_half, j, a,
               aggp_d, store):
    """RGCN aggregation for speaker-of-source a (relations 4a..4a+3).

    Computes agg_c[k] = sum_i Msk_{a,d}[i,k] * (x[i] @ W_rel[4a+2c+d])
    into 4 PSUM tiles [c][mb].  If store, evacuate+DMA to aggp_d[j];
    else return the live psum tiles."""
    # per-(a, mb) row multipliers  m_a = sp_mask_a / d
    ma = tiny.tile([128, 2, 1], F32, tag="ma")
    for mb in range(2):
        nc.vector.tensor_tensor(out=ma[:, mb, :], in0=rd_all[:, j, mb, :],
                                in1=spm_all[:, j, a, mb, :], op=ALU.mult)

    agg_ps = [[psAgg.tile([128, H], F32, tag=f"agg{c}{mb}", name=f"agg_ps{c}{mb}")
               for mb in range(2)] for c in range(2)]

    for dbit in range(2):
        msk = mskp.tile([128, 2, L], F32R, tag="msk")
        for mb in range(2):
            nc.vector.tensor_tensor(out=msk[:, mb, :], in0=u_all[:, j, mb, :],
                                    in1=btri[:, dbit, mb, :], op=ALU.mult)
            nc.vector.tensor_scalar(out=msk[:, mb, :], in0=msk[:, mb, :],
                                    scalar1=ma[:, mb, :], scalar2=None,
                                    op0=ALU.mult)
        for c in range(2):
            ri = 2 * c + dbit
            hs = []
            for ib in range(2):
                hp = psHall.tile([128, H], F32, tag="hallp")
                for kt in range(8):
                    nc.tensor.matmul(
                        out=hp, lhsT=_r(xT_t[:, kt, ds(ib * 128, 128)]),
                        rhs=_r(wrel_half[:, ri, kt, :]),
                        start=(kt == 0), stop=(kt == 7))
                h = hallsb.tile([128, H], F32R, tag="hall")
                cb.copy(out=h, in_=hp)
                hs.append(h)
            for kh in range(2):
                for ib in range(2):
                    nc.tensor.matmul(
                        out=agg_ps[c][kh],
                        lhsT=_r(msk[:, ib, ds(kh * 128, 128)]),
                        rhs=_r(hs[ib]),
                        start=(dbit == 0 and ib == 0),
                        stop=(dbit == 1 and ib == 1))

    if store:
        for c in range(2):
            for mb in range(2):
                t = aggsb.tile([128, H], F32, tag="aggsb")
                cb.copy(out=t, in_=agg_ps[c][mb])
                nc.scalar.dma_start(out=aggp_d[j, c, mb], in_=t)
        return None
    return agg_ps


# ---------------------------------------------------------------------------
# Host-side runner (axon / PJRT, 8 cores)
# ---------------------------------------------------------------------------
@functools.cache
def _runner():
    import jax
    from concourse import bass2jax
    from concourse.bass2jax import _bass_exec_p
    from jax.experimental.shard_map import shard_map
    from jax.sharding import Mesh, PartitionSpec

    bass2jax.install_neuronx_cc_hook()
    nc = _build_nc()

    part_name = nc.partition_id_tensor.name if nc.partition_id_tensor else None
    in_names, out_names, out_avals, zero_shapes = [], [], [], []
    for alloc in nc.m.functions[0].allocations:
        if not isinstance(alloc, mybir.MemoryLocationSet):
            continue
        name = alloc.memorylocations[0].name
        if alloc.kind == "ExternalInput":
            if name != part_name:
                in_names.append(name)
        elif alloc.kind == "ExternalOutput":
            out_names.append(name)
            shape = tuple(alloc.tensor_shape)
            dtype = mybir.dt.np(alloc.dtype)
            out_avals.append(jax.core.ShapedArray(shape, dtype))
            zero_shapes.append((shape, dtype))
    n_params = len(in_names)
    all_in_names = in_names + out_names
    if part_name is not None:
        all_in_names = all_in_names + [part_name]

    def _bass_body(*args):
        operands = list(args)
        if part_name is not None:
            operands.append(bass2jax.partition_id_tensor())
        outs = _bass_exec_p.bind(
            *operands,
            out_avals=tuple(out_avals),
            in_names=tuple(all_in_names),
            out_names=tuple(out_names),
            lowering_input_output_aliases=(),
            sim_require_finite=True,
            sim_require_nnan=True,
            nc=nc,
        )
        return tuple(outs)

    devices = jax.devices()[:NCORES]
    mesh = Mesh(np.asarray(devices), ("core",))
    n_outs = len(out_names)
    sharded = jax.jit(
        shard_map(_bass_body, mesh=mesh,
                  in_specs=(PartitionSpec("core"),) * (n_params + n_outs),
                  out_specs=(PartitionSpec("core"),) * n_outs,
                  check_rep=False),
        donate_argnums=tuple(range(n_params, n_params + n_outs)),
        keep_unused=True,
    )

    def run(concat_inputs):
        zeros = [np.zeros((NCORES * s[0], *s[1:]), d) for s, d in zero_shapes]
        out_arrs = sharded(*concat_inputs, *zeros)
        return {name: np.asarray(out_arrs[i]) for i, name in enumerate(out_names)}

    return run, in_names, out_names


def _pack_inputs(inputs, in_names):
    """Build the concatenated (8*shape0, ...) arrays in in_names order."""
    feats = np.asarray(inputs["features"], np.float32)   # [L, B, D]
    spks = np.asarray(inputs["speakers"], np.int32)      # [L, B]
    per_name = {}
    per_name["features"] = np.concatenate(
        [np.ascontiguousarray(feats[:, c * BPC:(c + 1) * BPC, :])
         for c in range(NCORES)], axis=0)
    per_name["speakers"] = np.concatenate(
        [np.ascontiguousarray(spks[:, c * BPC:(c + 1) * BPC])
         for c in range(NCORES)], axis=0)
    for name in in_names:
        if name in ("features", "speakers"):
            continue
        arr = np.ascontiguousarray(np.asarray(inputs[name], np.float32))
        per_name[name] = np.concatenate([arr] * NCORES, axis=0)
    return [per_name[n] for n in in_names]


def kernel(**inputs):
    run, in_names, out_names = _runner()
    concat = _pack_inputs(inputs, in_names)
    outs = run(concat)
    full = outs["out"]  # [8 * BPC * L, C] already conversation-major
    return full.reshape(B * L, C).astype(np.float32)


if __name__ == "__main__":
    rng = np.random.default_rng(0)
    fake = {
        "features": rng.standard_normal((L, B, D), dtype=np.float32),
        "Wscalar": rng.standard_normal((D, L), dtype=np.float32) * 0.02,
        "W_rel": rng.standard_normal((R, D, H), dtype=np.float32) * 0.02,
        "W_root": rng.standard_normal((D, H), dtype=np.float32) * 0.02,
        "b_rgcn": np.zeros(H, np.float32),
        "W_nbr": rng.standard_normal((H, H), dtype=np.float32) * 0.02,
        "W_self": rng.standard_normal((H, H), dtype=np.float32) * 0.02,
        "b_gc": np.zeros(H, np.float32),
        "W_match": rng.standard_normal((DH, DH), dtype=np.float32) * 0.02,
        "b_match": np.zeros(DH, np.float32),
        "W_lin": rng.standard_normal((DH, H), dtype=np.float32) * 0.02,
        "b_lin": np.zeros(H, np.float32),
        "W_fc": rng.standard_normal((H, C), dtype=np.float32) * 0.02,
        "b_fc": np.zeros(C, np.float32),
        "speakers": rng.integers(0, 2, (L, B)).astype(np.int32),
        "pair_i": np.zeros(1, np.int32),
        "pair_k": np.zeros(1, np.int32),
    }
    out = kernel(**fake)
    print("kernel output", out.shape, out.dtype, float(np.abs(out).max()))
